# revision 14
# baseline (speedup 1.0000x reference)
"""Tensor-parallel GQA attention block (dense_transformer) on 8 TRN2 NeuronCores.

Sharding: tensor parallel across heads — core c owns q-heads 4c..4c+3 and
kv-head c (GQA groups intact). Each core AllGathers softmax-normalized
per-head attention outputs y (bf16) and computes a 512-column slice of the
output projection; the host concatenates slices.

v3 pipeline: 512-row sequence-quarter granularity, with attention emitted one
projection phase late so every cross-engine handoff hides under matmuls:

    proj(0) proj(1) attn(0)+AG0 proj(2) attn(1)+AG1 proj(3) [wo load]
    attn(2)+AG2 attn(3)+AG3 outproj(0..3)

RoPE/evictions for quarter q run on ACT/DVE underneath proj(q+1)'s matmul
stream, so the PE enters attn(q) with zero dependency stalls and the HAM
clock gate never sees an idle window.  The four AllGathers (512 KB out /
4 MB in, ~35 us each) all complete well before their outproj consumers.

Device-side design:
  - operands live "contraction dim on partitions": xT [DIM,S], wqkvT
    [DIM,768], woT [DIM,512]; scores are computed transposed
    (S^T = K-tile^T @ Q^T); V is PE-transposed to [s,hd] per quarter.
  - RoPE via host-side head-dim permutation + single DVE stream_shuffle;
    softmax scale folded into wq; softmax skips max-subtraction (scores are
    O(10), exp cannot overflow f32).
  - causal masking is multiplicative post-exp: band01 = exp(cmask) (0/1
    image, built once on device) multiplies the exp tile on DVE in bf16 —
    cheaper than the additive f32 PSUM-read add, and off the ACT path.
    Fully-masked j-tiles are skipped entirely.
  - attention processes head pairs with one fused [P,2,SC] exp per j-tile
    (halves ACT instruction count) and a two-tile-deep software pipeline
    (PV matmuls of tile t-2 issue behind score matmuls of tile t), so the
    PE never waits on ACT/DVE.
  - softmax denominators come from a DVE running sum of the exp tiles plus
    one GpSimd cross-partition reduce per pair — no PE matmul slots at all.
    1/D via fast DVE reciprocal, broadcast on GpSimd, applied on DVE.
  - PSUM (8 banks): pss rotate E/A/B (2 banks each, 2-deep pipeline slack),
    C = psy pair accumulator / V-transpose scratch.  proj uses A/B/C;
    outproj ping-pongs A/B.
  - compute dtype: bf16 matmuls (fp32 PSUM), fp32 RoPE/softmax arithmetic.
"""

import ml_dtypes
import numpy as np

import concourse.bass as bass
import concourse.mybir as mybir
import concourse.tile as tile
from concourse import bacc
from concourse.bass_utils import run_bass_kernel_spmd

F32 = mybir.dt.float32
BF16 = mybir.dt.bfloat16
AF = mybir.ActivationFunctionType

N_CORES = 8
DIM = 4096
S = 2048
HEAD_DIM = 128
N_HEADS = 32
HPC = N_HEADS // N_CORES        # q heads per core = 4
P = 128
SC = 512                        # seq quarter (free dim of all matmuls)
N_Q = S // SC                   # 4
N_KTILE = DIM // P              # 32

SWAP16 = list(range(16, 32)) + list(range(16))   # per-quadrant 16-rotation


def build(debug_taps: bool = False):
    nc = bacc.Bacc(None, num_devices=N_CORES)

    xT = nc.declare_dram_parameter("xT", [DIM, S], BF16, isOutput=False)
    # fused qkv weights: [:, 0:512] q heads, [:, 512:640] k, [:, 640:768] v
    wqkvT = nc.declare_dram_parameter("wqkvT", [DIM, 768], BF16, isOutput=False)
    woT = nc.declare_dram_parameter("woT", [DIM, SC], BF16, isOutput=False)
    cosd = nc.declare_dram_parameter("cosd", [P, S], F32, isOutput=False)
    sins = nc.declare_dram_parameter("sins", [P, S], F32, isOutput=False)
    # causal band image: cmask[j, v] = 0 if (v - 1024) >= j else -1e9
    cmask = nc.declare_dram_parameter("cmask", [P, 2048], F32, isOutput=False)
    out = nc.dram_tensor("out", [S, SC], F32, kind="ExternalOutput")

    taps = {}
    if debug_taps:
        taps["qt"] = nc.dram_tensor("qt", [P, HPC, S], F32, kind="ExternalOutput")
        taps["kt"] = nc.dram_tensor("kt", [P, S], F32, kind="ExternalOutput")
        taps["vv"] = nc.dram_tensor("vv", [P, S // P, HEAD_DIM], F32,
                                    kind="ExternalOutput")
        taps["dd"] = nc.dram_tensor("dd", [HPC, S], F32, kind="ExternalOutput")
        taps["yl"] = nc.dram_tensor("yl", [P, HPC, S], BF16, kind="ExternalOutput")

    with tile.TileContext(nc) as tc:
        # PSUM bank budget (8 banks of 2KB/partition), 4 tags x 2 banks:
        #   proj:    A={q0,q1} B={q2,q3} C={k,v}
        #   attn:    E/A/B = pss 3-slot rotation, C = V-T scratch then psy
        #   outproj: A/B = pso ping-pong
        ps = tc.alloc_tile_pool(name="ps", bufs=1, space="PSUM")
        const = tc.alloc_tile_pool(name="const", bufs=1)
        pw = tc.alloc_tile_pool(name="pw", bufs=1, side="right")
        main = tc.alloc_tile_pool(name="main", bufs=1)
        stream = tc.alloc_tile_pool(name="stream", bufs=3)
        tmp = tc.alloc_tile_pool(name="tmp", bufs=2)
        dram = tc.alloc_tile_pool(name="dram", bufs=1, space="DRAM")

        # ---- constants ---------------------------------------------------
        ident = const.tile([P, P], F32)
        from concourse.masks import make_identity
        make_identity(nc, ident[:])
        ones_f = const.tile([P, 4], F32)
        nc.vector.memset(ones_f[:], 1.0)
        mask_sb = pw.tile([P, 2048], F32)
        band01 = const.tile([P, 2048], BF16)
        cos_sb = pw.tile([P, S], F32)
        sin_sb = pw.tile([P, S], F32)
        wqkv_sb = pw.tile([P, N_KTILE, 768], BF16)
        for k in range(N_KTILE):
            nc.scalar.dma_start(wqkv_sb[:, k, :], wqkvT[k * P:(k + 1) * P, :])

        kt_sb = main.tile([P, S], BF16)
        v_sb = main.tile([P, S // P, HEAD_DIM], BF16)
        qt_sb = main.tile([P, HPC, S], BF16)

        ybounce = [
            dram.tile([HPC * P, SC], BF16, name=f"ybounce{q}")
            for q in range(N_Q)
        ]
        ygather = [
            dram.tile([N_CORES * HPC * P, SC], BF16, addr_space="Shared",
                      name=f"ygather{q}")
            for q in range(N_Q)
        ]
        vts = {}
        raws = {}

        # ---- per-quarter segment emitters --------------------------------
        def proj(q):
            s_lo = q * SC
            psq01 = ps.tile([P, 2, SC], F32, tag="A", name=f"psq01_{q}")
            psq23 = ps.tile([P, 2, SC], F32, tag="B", name=f"psq23_{q}")
            pskv = ps.tile([P, 2, SC], F32, tag="C", name=f"pskv_{q}")
            for k in range(N_KTILE):
                xs = stream.tile([P, SC], BF16, tag="xs", bufs=10,
                                 name=f"xs{q}_{k}")
                nc.sync.dma_start(xs[:], xT[k * P:(k + 1) * P, s_lo:s_lo + SC])
                st = dict(start=(k == 0), stop=(k == N_KTILE - 1))
                for h in range(HPC):
                    dst = (psq01 if h < 2 else psq23)[:, h % 2, :]
                    nc.tensor.matmul(
                        dst, wqkv_sb[:, k, h * P:(h + 1) * P], xs[:], **st
                    )
                nc.tensor.matmul(pskv[:, 0, :], wqkv_sb[:, k, 512:640], xs[:], **st)
                nc.tensor.matmul(pskv[:, 1, :], wqkv_sb[:, k, 640:768], xs[:], **st)

            # fast merged PSUM evictions (frees banks for the next phase in
            # the PE queue), then RoPE from the SBUF copies.  The consumers
            # (attn(q)) are emitted a full proj phase later, so this whole
            # ACT+DVE chain hides under proj(q+1)'s matmul stream.
            raw01 = tmp.tile([P, 2, SC], F32, tag="rr2", bufs=2, name=f"r01_{q}")
            nc.scalar.copy(raw01[:], psq01[:])
            raw23 = tmp.tile([P, 2, SC], F32, tag="rr2", bufs=2, name=f"r23_{q}")
            nc.scalar.copy(raw23[:], psq23[:])
            rawk = tmp.tile([P, SC], F32, tag="rrk", bufs=2, name=f"rk_{q}")
            nc.scalar.copy(rawk[:], pskv[:, 0, :])
            vt = tmp.tile([P, SC], F32, tag="vt", bufs=2, name=f"vt{q}")
            nc.scalar.copy(vt[:], pskv[:, 1, :])
            vts[q] = vt

            raws[q] = (raw01, raw23, rawk)

        def rope(q, heads):
            """RoPE for quarter q, heads 0..3 = q-heads, 4 = k.  Emitted
            separately so the DVE queue order is [attn(q-1) ops, rope(q)] —
            rope never head-of-line-blocks the attention DVE work."""
            s_lo = q * SC
            raw01, raw23, rawk = raws[q]
            for h in heads:
                raw = rawk if h == HPC else \
                    (raw01 if h < 2 else raw23)[:, h % 2, :]
                dst = kt_sb[:, s_lo:s_lo + SC] if h == HPC \
                    else qt_sb[:, h, s_lo:s_lo + SC]
                qc = tmp.tile([P, SC], F32, tag="rqc", bufs=1, name=f"rq{q}_{h}")
                nc.vector.tensor_mul(qc[:], raw, cos_sb[:, s_lo:s_lo + SC])
                qsw = tmp.tile([P, SC], F32, tag="rqs", bufs=1, name=f"rs{q}_{h}")
                nc.vector.stream_shuffle(qsw[:], raw, SWAP16)
                nc.vector.tensor_mul(qsw[:], qsw[:], sin_sb[:, s_lo:s_lo + SC])
                nc.vector.tensor_add(dst, qc[:], qsw[:])

        def vtrans(q):
            # V transposes for this quarter (vt evicted back in proj(q))
            pstT = ps.tile([P, 2, SC], F32, tag="C", name=f"pst{q}")
            for t in range(4):
                nc.tensor.transpose(
                    pstT[:, t % 2, 0:P], vts[q][:, t * P:(t + 1) * P], ident[:]
                )
                nc.vector.tensor_copy(v_sb[:, q * 4 + t, :], pstT[:, t % 2, 0:P])

        def attn_pair(q, pair, prev_flush=None):
            n_j = 4 * q + 4
            s_lo = q * SC
            if True:
                h0 = 2 * pair
                psy_ref = [None]
                dacc = tmp.tile([P, 2, SC], F32, tag="dacc", bufs=2,
                                name=f"dacc{q}_{pair}")
                ptv = [None] * n_j
                held = [None]

                def pv(t):
                    if psy_ref[0] is None:
                        psy_ref[0] = ps.tile([P, 2, SC], F32, tag="C",
                                             name=f"psy_{q}_{pair}")
                    psy = psy_ref[0]
                    st = dict(start=(t == 0), stop=(t == n_j - 1))
                    for i in range(2):
                        nc.tensor.matmul(
                            psy[:, i, :], v_sb[:, t, :], ptv[t][:, i, :], **st)
                    ptv[t] = None

                for t in range(n_j):
                    pss = ps.tile([P, 2, SC], F32, tag="EAB"[t % 3],
                                  name=f"pss{q}_{pair}_{t}")
                    for i in range(2):
                        nc.tensor.matmul(
                            pss[:, i, :],
                            kt_sb[:, t * P:(t + 1) * P],
                            qt_sb[:, h0 + i, s_lo:s_lo + SC],
                            start=True, stop=True,
                        )
                    pt2 = tmp.tile([P, 2, SC], BF16, tag="pt", bufs=3,
                                   name=f"pt{q}_{pair}_{t}")
                    nc.scalar.activation(pt2[:], pss[:], AF.Exp)
                    d = t - 4 * q
                    if d >= 0:
                        ptm = tmp.tile([P, 2, SC], BF16, tag="ptm", bufs=3,
                                       name=f"ptm{q}_{pair}_{t}")
                        for i in range(2):
                            nc.vector.tensor_mul(
                                ptm[:, i, :], pt2[:, i, :],
                                band01[:, 1024 - P * d:1536 - P * d],
                            )
                        use = ptm
                    else:
                        use = pt2
                    ptv[t] = use
                    # D accumulation: non-diagonal tiles pair up in bf16
                    # (half-rate DVE cost), diagonal/odd tiles add directly
                    n_nd = 4 * q             # non-diag tiles; even count
                    if d < 0 and t % 2 == 0:
                        held[0] = use
                    elif d < 0 and t % 2 == 1:
                        dsum = tmp.tile([P, 2, SC], BF16, tag="dsum", bufs=2,
                                        name=f"ds{q}_{pair}_{t}")
                        nc.vector.tensor_add(dsum[:], held[0][:, :, :], use[:])
                        held[0] = None
                        if t == 1:
                            nc.vector.tensor_copy(dacc[:], dsum[:])
                        else:
                            nc.vector.tensor_add(dacc[:], dacc[:], dsum[:])
                    elif t == 0:
                        nc.vector.tensor_copy(dacc[:], use[:])
                    else:
                        nc.vector.tensor_add(dacc[:], dacc[:], use[:])
                    if t == 1 and prev_flush is not None:
                        prev_flush()
                        prev_flush = None
                    if t >= 2:
                        pv(t - 2)
                if prev_flush is not None:
                    prev_flush()

            state = {}

            def flush():
                # tail PV matmuls (issued behind the next pair's first score
                # matmuls so their exp never stalls the PE), evict psy (ACT),
                # then reduce D across partitions with two cheap f32
                # ones-matmuls into the freed C banks (the GpSimd
                # partition-reduce took 7.4us and stalled the DVE chain).
                pv(n_j - 2)
                pv(n_j - 1)
                psy = psy_ref[0]
                ysb2 = tmp.tile([P, 2, SC], F32, tag="ysb", bufs=1,
                                name=f"ysb{q}_{pair}")
                nc.scalar.copy(ysb2[:], psy[:])
                psd = ps.tile([1, 2, SC], F32, tag="C", name=f"psd{q}_{pair}")
                for i in range(2):
                    nc.tensor.matmul(psd[:, i, :], ones_f[:, 0:1],
                                     dacc[:, i, :], start=True, stop=True)
                d2 = tmp.tile([1, 2, SC], F32, tag="d2", bufs=1,
                              name=f"d2{q}_{pair}")
                nc.scalar.copy(d2[:], psd[:])
                state["ysb2"], state["d2"] = ysb2, d2

            def norm():
                # deferred so the reduce/evict chain lands where the PE has
                # independent work (next pair / next proj)
                ysb2, d2 = state["ysb2"], state["d2"]
                rc2 = tmp.tile([1, 2, SC], F32, tag="rc2", bufs=1,
                               name=f"rc2{q}_{pair}")
                nc.vector.reciprocal_approx_fast(rc2[:], d2[:])
                rb2 = tmp.tile([P, 2, SC], F32, tag="rb2", bufs=1,
                               name=f"rb2{q}_{pair}")
                nc.gpsimd.partition_broadcast(rb2[:], rc2[:])
                yp2 = tmp.tile([P, 2, SC], BF16, tag="yp", bufs=1,
                               name=f"yp{q}_{pair}")
                nc.vector.tensor_mul(yp2[:], ysb2[:], rb2[:])
                nc.gpsimd.dma_start(
                    ybounce[q][h0 * P:(h0 + 2) * P, :]
                    .rearrange("(i p) m -> p i m", p=P),
                    yp2[:],
                )
                if debug_taps:
                    for i in range(2):
                        h = h0 + i
                        nc.sync.dma_start(taps["yl"][:, h, s_lo:s_lo + SC],
                                          yp2[:, i, :])
                        nc.sync.dma_start(taps["dd"][h:h + 1, s_lo:s_lo + SC],
                                          d2[:, i, :])

            return flush, norm

        def ag(q):
            nc.gpsimd.collective_compute(
                "AllGather",
                mybir.AluOpType.bypass,
                replica_groups=[list(range(N_CORES))],
                ins=[ybounce[q][:]],
                outs=[ygather[q][:]],
            )

        yg_tiles = {}

        def load_yg(q):
            yg_tiles[q] = pyg.tile([P, N_KTILE, SC], BF16, tag="yg",
                                   bufs=2, name=f"yg{q}")
            for j in range(4):
                nc.sync.dma_start(
                    yg_tiles[q][:, 8 * j:8 * (j + 1), :],
                    ygather[q][j * 8 * P:(j + 1) * 8 * P, :]
                    .rearrange("(t p) m -> p t m", p=P),
                )

        def outproj(q):
            g_lo = q * SC
            yg = yg_tiles[q]
            for st_i in range(4):
                pso = ps.tile(
                    [P, SC], F32, tag=("A" if st_i % 2 == 0 else "B"),
                    name=f"pso{q}_{st_i}",
                )
                for kt in range(N_KTILE):
                    nc.tensor.matmul(
                        pso[:],
                        yg[:, kt, st_i * P:(st_i + 1) * P],
                        wo_sb[:, kt, :],
                        start=(kt == 0), stop=(kt == N_KTILE - 1),
                    )
                ob = tmp.tile([P, SC], F32, tag="ob", name=f"ob{q}_{st_i}")
                nc.scalar.copy(ob[:], pso[:])
                nc.gpsimd.dma_start(
                    out[g_lo + st_i * P:g_lo + (st_i + 1) * P, :], ob[:]
                )

        # ---- pipelined emission ------------------------------------------
        # DVE queue order is [rope(q), attn(q) ops, rope(q+1), ...]: each
        # rope chunk becomes ready exactly when the preceding attention's
        # DVE work drains, so nothing head-of-line blocks.  rope(3) is
        # split around attn(2)'s pairs for the same reason.
        def attn(q):
            vtrans(q)
            fl0, no0 = attn_pair(q, 0)
            if q == 2:
                rope(3, [0, 1])
            fl1, no1 = attn_pair(q, 1, fl0)
            if q == 2:
                rope(3, [4, 2, 3])
            no0()
            fl1()
            no1()
            ag(q)

        proj(0)
        nc.sync.dma_start(cos_sb[:], cosd[:])
        nc.sync.dma_start(sin_sb[:], sins[:])
        nc.sync.dma_start(mask_sb[:], cmask[:])
        nc.scalar.activation(band01[:], mask_sb[:], AF.Exp)
        proj(1)
        rope(0, [0, 1, 2, 3, 4])
        attn(0)
        proj(2)
        rope(1, [0, 1, 2, 3, 4])
        attn(1)
        proj(3)
        rope(2, [0, 1, 2, 3, 4])
        attn(2)
        pw.release()
        pw2 = tc.alloc_tile_pool(name="pw2", bufs=1, side="right")
        wo_sb = pw2.tile([P, N_KTILE, SC], BF16)
        nc.sync.dma_start(wo_sb[:], woT.rearrange("(t p) m -> p t m", p=P))
        attn(3)

        if debug_taps:
            nc.sync.dma_start(taps["qt"][:], qt_sb[:])
            nc.sync.dma_start(taps["kt"][:], kt_sb[:])
            nc.sync.dma_start(taps["vv"][:], v_sb[:])

        pyg = tc.alloc_tile_pool(name="pyg", bufs=1, side="right")
        load_yg(0)
        load_yg(1)
        outproj(0)
        load_yg(2)
        outproj(1)
        load_yg(3)
        outproj(2)
        outproj(3)

        for pool in (pyg, pw2, dram, tmp, stream, main, const, ps):
            pool.release()

    nc.compile()
    return nc


# ---------------------------------------------------------------------------
# host-side prep / unshard
# ---------------------------------------------------------------------------

def _perm128():
    """head-dim permutation: pair i=(16q+j) -> even at 32q+j, odd at 32q+16+j."""
    order = np.empty(128, dtype=np.int64)
    for i in range(64):
        q, j = i // 16, i % 16
        order[32 * q + j] = 2 * i
        order[32 * q + 16 + j] = 2 * i + 1
    return order


def _host_prep(x, freqs_cis, wq, wk, wv, wo):
    order = _perm128()
    xT = np.ascontiguousarray(x[0].T)                       # [DIM, S]
    scale = np.float32(1.0 / np.sqrt(HEAD_DIM))

    cosT = np.ascontiguousarray(freqs_cis[:, :, 0].T)       # [64, S]
    sinT = np.ascontiguousarray(freqs_cis[:, :, 1].T)
    cosd = np.empty((P, S), dtype=np.float32)
    sins = np.empty((P, S), dtype=np.float32)
    for q in range(4):
        cosd[32 * q:32 * q + 16] = cosT[16 * q:16 * q + 16]
        cosd[32 * q + 16:32 * q + 32] = cosT[16 * q:16 * q + 16]
        sins[32 * q:32 * q + 16] = -sinT[16 * q:16 * q + 16]
        sins[32 * q + 16:32 * q + 32] = sinT[16 * q:16 * q + 16]

    vv = np.arange(2048)[None, :]
    jj = np.arange(P)[:, None]
    cmask = np.where(vv - 1024 >= jj, np.float32(0.0), np.float32(-1e9))
    cmask = np.ascontiguousarray(cmask, dtype=np.float32)

    xT16 = xT.astype(ml_dtypes.bfloat16)
    in_maps = []
    for c in range(N_CORES):
        wq_c = wq[c * 512:(c + 1) * 512].reshape(HPC, 128, DIM)[:, order, :]
        wq_c = (wq_c.reshape(512, DIM) * scale).astype(np.float32)
        wk_c = wk[c * 128:(c + 1) * 128][order]
        wv_c = wv[c * 128:(c + 1) * 128]
        wqkv_c = np.concatenate([wq_c, wk_c, wv_c], axis=0)
        wo_c = wo[c * 512:(c + 1) * 512]
        in_maps.append({
            "xT": xT16,
            "wqkvT": np.ascontiguousarray(wqkv_c.T).astype(ml_dtypes.bfloat16),
            "woT": np.ascontiguousarray(wo_c.T).astype(ml_dtypes.bfloat16),
            "cosd": cosd,
            "sins": sins,
            "cmask": cmask,
        })
    return in_maps


_NC_CACHE = {}


def get_nc(debug_taps=False):
    key = bool(debug_taps)
    if key not in _NC_CACHE:
        _NC_CACHE[key] = build(debug_taps=key)
    return _NC_CACHE[key]


def kernel(x, freqs_cis, mask, wq, wk, wv, wo, _trace=False, _debug_taps=False,
           _warmup=False):
    in_maps = _host_prep(x, freqs_cis, wq, wk, wv, wo)
    nc = get_nc(_debug_taps)
    if _warmup:
        run_bass_kernel_spmd(
            nc, in_maps, core_ids=list(range(N_CORES)), trace=False
        )
    res = run_bass_kernel_spmd(
        nc, in_maps, core_ids=list(range(N_CORES)), trace=_trace
    )
    full = np.concatenate([res.results[c]["out"] for c in range(N_CORES)], axis=1)
    out = full.reshape(1, S, DIM).astype(np.float32)
    if _trace or _debug_taps:
        kernel.last_results = res
    return out


# revision 15
# speedup vs baseline: 1.0675x; 1.0675x over previous
"""Tensor-parallel GQA attention block (dense_transformer) on 8 TRN2 NeuronCores.

Sharding: tensor parallel across heads — core c owns q-heads 4c..4c+3 and
kv-head c (GQA groups intact). Each core AllGathers softmax-normalized
per-head attention outputs y (bf16) and computes a 512-column slice of the
output projection; the host concatenates slices.

v3 pipeline: 512-row sequence-quarter granularity, with attention emitted one
projection phase late so every cross-engine handoff hides under matmuls:

    proj(0) proj(1) attn(0)+AG0 proj(2) attn(1)+AG1 proj(3) [wo load]
    attn(2)+AG2 attn(3)+AG3 outproj(0..3)

RoPE/evictions for quarter q run on ACT/DVE underneath proj(q+1)'s matmul
stream, so the PE enters attn(q) with zero dependency stalls and the HAM
clock gate never sees an idle window.  The four AllGathers (512 KB out /
4 MB in, ~35 us each) all complete well before their outproj consumers.

Device-side design:
  - operands live "contraction dim on partitions": xT [DIM,S], wqkvT
    [DIM,768], woT [DIM,512]; scores are computed transposed
    (S^T = K-tile^T @ Q^T); V is PE-transposed to [s,hd] per quarter.
  - RoPE via host-side head-dim permutation + single DVE stream_shuffle;
    softmax scale folded into wq; softmax skips max-subtraction (scores are
    O(10), exp cannot overflow f32).
  - causal masking is multiplicative post-exp: band01 = exp(cmask) (0/1
    image, built once on device) multiplies the exp tile on DVE in bf16 —
    cheaper than the additive f32 PSUM-read add, and off the ACT path.
    Fully-masked j-tiles are skipped entirely.
  - attention processes head pairs with one fused [P,2,SC] exp per j-tile
    (halves ACT instruction count) and a two-tile-deep software pipeline
    (PV matmuls of tile t-2 issue behind score matmuls of tile t), so the
    PE never waits on ACT/DVE.
  - softmax denominators come from a DVE running sum of the exp tiles plus
    one GpSimd cross-partition reduce per pair — no PE matmul slots at all.
    1/D via fast DVE reciprocal, broadcast on GpSimd, applied on DVE.
  - PSUM (8 banks): pss rotate E/A/B (2 banks each, 2-deep pipeline slack),
    C = psy pair accumulator / V-transpose scratch.  proj uses A/B/C;
    outproj ping-pongs A/B.
  - compute dtype: bf16 matmuls (fp32 PSUM), fp32 RoPE/softmax arithmetic.
"""

import ml_dtypes
import numpy as np

import concourse.bass as bass
import concourse.mybir as mybir
import concourse.tile as tile
from concourse import bacc
from concourse.bass_utils import run_bass_kernel_spmd

F32 = mybir.dt.float32
BF16 = mybir.dt.bfloat16
AF = mybir.ActivationFunctionType

N_CORES = 8
DIM = 4096
S = 2048
HEAD_DIM = 128
N_HEADS = 32
HPC = N_HEADS // N_CORES        # q heads per core = 4
P = 128
SC = 512                        # seq quarter (free dim of all matmuls)
N_Q = S // SC                   # 4
N_KTILE = DIM // P              # 32

SWAP16 = list(range(16, 32)) + list(range(16))   # per-quadrant 16-rotation


def build(debug_taps: bool = False):
    nc = bacc.Bacc(None, num_devices=N_CORES)

    xT = nc.declare_dram_parameter("xT", [DIM, S], BF16, isOutput=False)
    # fused qkv weights: [:, 0:512] q heads, [:, 512:640] k, [:, 640:768] v
    wqkvT = nc.declare_dram_parameter("wqkvT", [DIM, 768], BF16, isOutput=False)
    woT = nc.declare_dram_parameter("woT", [DIM, SC], BF16, isOutput=False)
    cosd = nc.declare_dram_parameter("cosd", [P, S], F32, isOutput=False)
    sins = nc.declare_dram_parameter("sins", [P, S], F32, isOutput=False)
    # causal band image: cmask[j, v] = 0 if (v - 1024) >= j else -1e9
    cmask = nc.declare_dram_parameter("cmask", [P, 2048], F32, isOutput=False)
    out = nc.dram_tensor("out", [S, SC], F32, kind="ExternalOutput")

    taps = {}
    if debug_taps:
        taps["qt"] = nc.dram_tensor("qt", [P, HPC, S], F32, kind="ExternalOutput")
        taps["kt"] = nc.dram_tensor("kt", [P, S], F32, kind="ExternalOutput")
        taps["vv"] = nc.dram_tensor("vv", [P, S // P, HEAD_DIM], F32,
                                    kind="ExternalOutput")
        taps["dd"] = nc.dram_tensor("dd", [HPC, S], F32, kind="ExternalOutput")
        taps["yl"] = nc.dram_tensor("yl", [P, HPC, S], BF16, kind="ExternalOutput")

    with tile.TileContext(nc) as tc:
        # PSUM bank budget (8 banks of 2KB/partition), 4 tags x 2 banks:
        #   proj:    A={q0,q1} B={q2,q3} C={k,v}
        #   attn:    E/A/B = pss 3-slot rotation, C = V-T scratch then psy
        #   outproj: A/B = pso ping-pong
        ps = tc.alloc_tile_pool(name="ps", bufs=1, space="PSUM")
        const = tc.alloc_tile_pool(name="const", bufs=1)
        pw = tc.alloc_tile_pool(name="pw", bufs=1, side="right")
        main = tc.alloc_tile_pool(name="main", bufs=1)
        stream = tc.alloc_tile_pool(name="stream", bufs=3)
        tmp = tc.alloc_tile_pool(name="tmp", bufs=2)
        dram = tc.alloc_tile_pool(name="dram", bufs=1, space="DRAM")

        # ---- constants ---------------------------------------------------
        ident = const.tile([P, P], F32)
        from concourse.masks import make_identity
        make_identity(nc, ident[:])
        ones_f = const.tile([P, 4], F32)
        nc.vector.memset(ones_f[:], 1.0)
        ones_b = const.tile([P, 4], BF16)
        nc.scalar.copy(ones_b[:], ones_f[:])
        mask_sb = pw.tile([P, 2048], F32)
        band01 = const.tile([P, 2048], BF16)
        cos_sb = pw.tile([P, S], F32)
        sin_sb = pw.tile([P, S], F32)
        wqkv_sb = pw.tile([P, N_KTILE, 768], BF16)
        for k in range(N_KTILE):
            nc.scalar.dma_start(wqkv_sb[:, k, :], wqkvT[k * P:(k + 1) * P, :])

        kt_sb = main.tile([P, S], BF16)
        v_sb = main.tile([P, S // P, HEAD_DIM], BF16)
        qt_sb = main.tile([P, HPC, S], BF16)

        ybounce = [
            dram.tile([HPC * P, SC], BF16, name=f"ybounce{q}")
            for q in range(N_Q)
        ]
        ygather = [
            dram.tile([N_CORES * HPC * P, SC], BF16, addr_space="Shared",
                      name=f"ygather{q}")
            for q in range(N_Q)
        ]
        vts = {}
        raws = {}

        # ---- per-quarter segment emitters --------------------------------
        def proj(q):
            s_lo = q * SC
            psq01 = ps.tile([P, 2, SC], F32, tag="A", name=f"psq01_{q}")
            psq23 = ps.tile([P, 2, SC], F32, tag="B", name=f"psq23_{q}")
            pskv = ps.tile([P, 2, SC], F32, tag="C", name=f"pskv_{q}")
            for k in range(N_KTILE):
                xs = stream.tile([P, SC], BF16, tag="xs", bufs=10,
                                 name=f"xs{q}_{k}")
                nc.sync.dma_start(xs[:], xT[k * P:(k + 1) * P, s_lo:s_lo + SC])
                st = dict(start=(k == 0), stop=(k == N_KTILE - 1))
                for h in range(HPC):
                    dst = (psq01 if h < 2 else psq23)[:, h % 2, :]
                    nc.tensor.matmul(
                        dst, wqkv_sb[:, k, h * P:(h + 1) * P], xs[:], **st
                    )
                nc.tensor.matmul(pskv[:, 0, :], wqkv_sb[:, k, 512:640], xs[:], **st)
                nc.tensor.matmul(pskv[:, 1, :], wqkv_sb[:, k, 640:768], xs[:], **st)

            # fast merged PSUM evictions (frees banks for the next phase in
            # the PE queue), then RoPE from the SBUF copies.  The consumers
            # (attn(q)) are emitted a full proj phase later, so this whole
            # ACT+DVE chain hides under proj(q+1)'s matmul stream.
            raw01 = tmp.tile([P, 2, SC], F32, tag="rr2", bufs=2, name=f"r01_{q}")
            nc.scalar.copy(raw01[:], psq01[:])
            raw23 = tmp.tile([P, 2, SC], F32, tag="rr2", bufs=2, name=f"r23_{q}")
            nc.scalar.copy(raw23[:], psq23[:])
            rawk = tmp.tile([P, SC], F32, tag="rrk", bufs=2, name=f"rk_{q}")
            nc.scalar.copy(rawk[:], pskv[:, 0, :])
            vt = tmp.tile([P, SC], F32, tag="vt", bufs=2, name=f"vt{q}")
            nc.scalar.copy(vt[:], pskv[:, 1, :])
            vts[q] = vt

            raws[q] = (raw01, raw23, rawk)

        def rope(q, heads):
            """RoPE for quarter q, heads 0..3 = q-heads, 4 = k.  Emitted
            separately so the DVE queue order is [attn(q-1) ops, rope(q)] —
            rope never head-of-line-blocks the attention DVE work."""
            s_lo = q * SC
            raw01, raw23, rawk = raws[q]
            for h in heads:
                raw = rawk if h == HPC else \
                    (raw01 if h < 2 else raw23)[:, h % 2, :]
                dst = kt_sb[:, s_lo:s_lo + SC] if h == HPC \
                    else qt_sb[:, h, s_lo:s_lo + SC]
                qc = tmp.tile([P, SC], F32, tag="rqc", bufs=1, name=f"rq{q}_{h}")
                nc.vector.tensor_mul(qc[:], raw, cos_sb[:, s_lo:s_lo + SC])
                qsw = tmp.tile([P, SC], F32, tag="rqs", bufs=1, name=f"rs{q}_{h}")
                nc.vector.stream_shuffle(qsw[:], raw, SWAP16)
                nc.vector.tensor_mul(qsw[:], qsw[:], sin_sb[:, s_lo:s_lo + SC])
                nc.vector.tensor_add(dst, qc[:], qsw[:])

        def vtrans(q):
            # V transposes for this quarter (vt evicted back in proj(q))
            pstT = ps.tile([P, 2, SC], F32, tag="C", name=f"pst{q}")
            for t in range(4):
                nc.tensor.transpose(
                    pstT[:, t % 2, 0:P], vts[q][:, t * P:(t + 1) * P], ident[:]
                )
                nc.vector.tensor_copy(v_sb[:, q * 4 + t, :], pstT[:, t % 2, 0:P])

        def attn_pair(q, pair, prev_flush=None):
            n_j = 4 * q + 4
            s_lo = q * SC
            if True:
                h0 = 2 * pair
                psy_ref = [None]
                dacc = tmp.tile([P, 2, SC], F32, tag="dacc", bufs=2,
                                name=f"dacc{q}_{pair}")
                ptv = [None] * n_j
                held = [None]

                def pv(t):
                    if psy_ref[0] is None:
                        psy_ref[0] = ps.tile([P, 2, SC], F32, tag="C",
                                             name=f"psy_{q}_{pair}")
                    psy = psy_ref[0]
                    st = dict(start=(t == 0), stop=(t == n_j - 1))
                    for i in range(2):
                        nc.tensor.matmul(
                            psy[:, i, :], v_sb[:, t, :], ptv[t][:, i, :], **st)
                    ptv[t] = None

                for t in range(n_j):
                    pss = ps.tile([P, 2, SC], F32, tag="EAB"[t % 3],
                                  name=f"pss{q}_{pair}_{t}")
                    for i in range(2):
                        nc.tensor.matmul(
                            pss[:, i, :],
                            kt_sb[:, t * P:(t + 1) * P],
                            qt_sb[:, h0 + i, s_lo:s_lo + SC],
                            start=True, stop=True,
                        )
                    pt2 = tmp.tile([P, 2, SC], BF16, tag="pt", bufs=3,
                                   name=f"pt{q}_{pair}_{t}")
                    nc.scalar.activation(pt2[:], pss[:], AF.Exp)
                    d = t - 4 * q
                    if d >= 0:
                        ptm = tmp.tile([P, 2, SC], BF16, tag="ptm", bufs=3,
                                       name=f"ptm{q}_{pair}_{t}")
                        for i in range(2):
                            nc.vector.tensor_mul(
                                ptm[:, i, :], pt2[:, i, :],
                                band01[:, 1024 - P * d:1536 - P * d],
                            )
                        use = ptm
                    else:
                        use = pt2
                    ptv[t] = use
                    # D accumulation: tiles pair up in bf16 (half-rate DVE
                    # cost), f32 running sum every second tile (n_j is even)
                    if t % 2 == 0:
                        held[0] = use
                    else:
                        dsum = tmp.tile([P, 2, SC], BF16, tag="dsum", bufs=2,
                                        name=f"ds{q}_{pair}_{t}")
                        nc.vector.tensor_add(dsum[:], held[0][:, :, :], use[:])
                        held[0] = None
                        if t == 1:
                            nc.vector.tensor_copy(dacc[:], dsum[:])
                        else:
                            nc.vector.tensor_add(dacc[:], dacc[:], dsum[:])
                    if t == 1 and prev_flush is not None:
                        prev_flush()
                        prev_flush = None
                    if t >= 2:
                        pv(t - 2)
                if prev_flush is not None:
                    prev_flush()

            state = {}

            def flush():
                # tail PV matmuls (issued behind the next pair's first score
                # matmuls so their exp never stalls the PE), evict psy (ACT),
                # then reduce D across partitions with two cheap f32
                # ones-matmuls into the freed C banks (the GpSimd
                # partition-reduce took 7.4us and stalled the DVE chain).
                pv(n_j - 2)
                pv(n_j - 1)
                psy = psy_ref[0]
                ysb2 = tmp.tile([P, 2, SC], F32, tag="ysb", bufs=1,
                                name=f"ysb{q}_{pair}")
                nc.scalar.copy(ysb2[:], psy[:])
                dac16 = tmp.tile([P, 2, SC], BF16, tag="da16", bufs=1,
                                 name=f"da16{q}_{pair}")
                nc.vector.tensor_copy(dac16[:], dacc[:])
                psd = ps.tile([1, 2, SC], F32, tag="C", name=f"psd{q}_{pair}")
                for i in range(2):
                    nc.tensor.matmul(psd[:, i, :], ones_b[:, 0:1],
                                     dac16[:, i, :], start=True, stop=True)
                d2 = tmp.tile([1, 2, SC], F32, tag="d2", bufs=1,
                              name=f"d2{q}_{pair}")
                nc.scalar.copy(d2[:], psd[:])
                state["ysb2"], state["d2"] = ysb2, d2

            def norm():
                # deferred so the reduce/evict chain lands where the PE has
                # independent work (next pair / next proj)
                ysb2, d2 = state["ysb2"], state["d2"]
                rc2 = tmp.tile([1, 2, SC], F32, tag="rc2", bufs=1,
                               name=f"rc2{q}_{pair}")
                nc.vector.reciprocal_approx_fast(rc2[:], d2[:])
                rb2 = tmp.tile([P, 2, SC], F32, tag="rb2", bufs=1,
                               name=f"rb2{q}_{pair}")
                nc.gpsimd.partition_broadcast(rb2[:], rc2[:])
                yp2 = tmp.tile([P, 2, SC], BF16, tag="yp", bufs=1,
                               name=f"yp{q}_{pair}")
                nc.vector.tensor_mul(yp2[:], ysb2[:], rb2[:])
                nc.gpsimd.dma_start(
                    ybounce[q][h0 * P:(h0 + 2) * P, :]
                    .rearrange("(i p) m -> p i m", p=P),
                    yp2[:],
                )
                if debug_taps:
                    for i in range(2):
                        h = h0 + i
                        nc.sync.dma_start(taps["yl"][:, h, s_lo:s_lo + SC],
                                          yp2[:, i, :])
                        nc.sync.dma_start(taps["dd"][h:h + 1, s_lo:s_lo + SC],
                                          d2[:, i, :])

            return flush, norm

        def ag(q):
            nc.gpsimd.collective_compute(
                "AllGather",
                mybir.AluOpType.bypass,
                replica_groups=[list(range(N_CORES))],
                ins=[ybounce[q][:]],
                outs=[ygather[q][:]],
            )

        yg_tiles = {}

        def load_yg(q):
            yg_tiles[q] = pyg.tile([P, N_KTILE, SC], BF16, tag="yg",
                                   bufs=2, name=f"yg{q}")
            for j in range(4):
                nc.sync.dma_start(
                    yg_tiles[q][:, 8 * j:8 * (j + 1), :],
                    ygather[q][j * 8 * P:(j + 1) * 8 * P, :]
                    .rearrange("(t p) m -> p t m", p=P),
                )

        def outproj(q):
            g_lo = q * SC
            yg = yg_tiles[q]
            for st_i in range(4):
                pso = ps.tile(
                    [P, SC], F32, tag=("A" if st_i % 2 == 0 else "B"),
                    name=f"pso{q}_{st_i}",
                )
                for kt in range(N_KTILE):
                    nc.tensor.matmul(
                        pso[:],
                        yg[:, kt, st_i * P:(st_i + 1) * P],
                        wo_sb[:, kt, :],
                        start=(kt == 0), stop=(kt == N_KTILE - 1),
                    )
                ob = tmp.tile([P, SC], F32, tag="ob", name=f"ob{q}_{st_i}")
                nc.scalar.copy(ob[:], pso[:])
                nc.gpsimd.dma_start(
                    out[g_lo + st_i * P:g_lo + (st_i + 1) * P, :], ob[:]
                )

        # ---- pipelined emission ------------------------------------------
        # DVE queue order is [rope(q), attn(q) ops, rope(q+1), ...]: each
        # rope chunk becomes ready exactly when the preceding attention's
        # DVE work drains, so nothing head-of-line blocks.  rope(3) is
        # split around attn(2)'s pairs for the same reason.
        def attn(q):
            vtrans(q)
            fl0, no0 = attn_pair(q, 0)
            if q == 2:
                rope(3, [0, 1])
            fl1, no1 = attn_pair(q, 1, fl0)
            if q == 2:
                rope(3, [4, 2, 3])
            no0()
            fl1()
            no1()
            ag(q)

        proj(0)
        nc.sync.dma_start(cos_sb[:], cosd[:])
        nc.sync.dma_start(sin_sb[:], sins[:])
        nc.sync.dma_start(mask_sb[:], cmask[:])
        nc.scalar.activation(band01[:], mask_sb[:], AF.Exp)
        proj(1)
        rope(0, [0, 1, 2, 3, 4])
        attn(0)
        proj(2)
        rope(1, [0, 1, 2, 3, 4])
        attn(1)
        proj(3)
        rope(2, [0, 1, 2, 3, 4])
        attn(2)
        pw.release()
        pw2 = tc.alloc_tile_pool(name="pw2", bufs=1, side="right")
        wo_sb = pw2.tile([P, N_KTILE, SC], BF16)
        nc.sync.dma_start(wo_sb[:], woT.rearrange("(t p) m -> p t m", p=P))
        attn(3)

        if debug_taps:
            nc.sync.dma_start(taps["qt"][:], qt_sb[:])
            nc.sync.dma_start(taps["kt"][:], kt_sb[:])
            nc.sync.dma_start(taps["vv"][:], v_sb[:])

        pyg = tc.alloc_tile_pool(name="pyg", bufs=1, side="right")
        load_yg(0)
        load_yg(1)
        outproj(0)
        load_yg(2)
        outproj(1)
        load_yg(3)
        outproj(2)
        outproj(3)

        for pool in (pyg, pw2, dram, tmp, stream, main, const, ps):
            pool.release()

    nc.compile()
    return nc


# ---------------------------------------------------------------------------
# host-side prep / unshard
# ---------------------------------------------------------------------------

def _perm128():
    """head-dim permutation: pair i=(16q+j) -> even at 32q+j, odd at 32q+16+j."""
    order = np.empty(128, dtype=np.int64)
    for i in range(64):
        q, j = i // 16, i % 16
        order[32 * q + j] = 2 * i
        order[32 * q + 16 + j] = 2 * i + 1
    return order


def _host_prep(x, freqs_cis, wq, wk, wv, wo):
    order = _perm128()
    xT = np.ascontiguousarray(x[0].T)                       # [DIM, S]
    scale = np.float32(1.0 / np.sqrt(HEAD_DIM))

    cosT = np.ascontiguousarray(freqs_cis[:, :, 0].T)       # [64, S]
    sinT = np.ascontiguousarray(freqs_cis[:, :, 1].T)
    cosd = np.empty((P, S), dtype=np.float32)
    sins = np.empty((P, S), dtype=np.float32)
    for q in range(4):
        cosd[32 * q:32 * q + 16] = cosT[16 * q:16 * q + 16]
        cosd[32 * q + 16:32 * q + 32] = cosT[16 * q:16 * q + 16]
        sins[32 * q:32 * q + 16] = -sinT[16 * q:16 * q + 16]
        sins[32 * q + 16:32 * q + 32] = sinT[16 * q:16 * q + 16]

    vv = np.arange(2048)[None, :]
    jj = np.arange(P)[:, None]
    cmask = np.where(vv - 1024 >= jj, np.float32(0.0), np.float32(-1e9))
    cmask = np.ascontiguousarray(cmask, dtype=np.float32)

    xT16 = xT.astype(ml_dtypes.bfloat16)
    in_maps = []
    for c in range(N_CORES):
        wq_c = wq[c * 512:(c + 1) * 512].reshape(HPC, 128, DIM)[:, order, :]
        wq_c = (wq_c.reshape(512, DIM) * scale).astype(np.float32)
        wk_c = wk[c * 128:(c + 1) * 128][order]
        wv_c = wv[c * 128:(c + 1) * 128]
        wqkv_c = np.concatenate([wq_c, wk_c, wv_c], axis=0)
        wo_c = wo[c * 512:(c + 1) * 512]
        in_maps.append({
            "xT": xT16,
            "wqkvT": np.ascontiguousarray(wqkv_c.T).astype(ml_dtypes.bfloat16),
            "woT": np.ascontiguousarray(wo_c.T).astype(ml_dtypes.bfloat16),
            "cosd": cosd,
            "sins": sins,
            "cmask": cmask,
        })
    return in_maps


_NC_CACHE = {}


def get_nc(debug_taps=False):
    key = bool(debug_taps)
    if key not in _NC_CACHE:
        _NC_CACHE[key] = build(debug_taps=key)
    return _NC_CACHE[key]


def kernel(x, freqs_cis, mask, wq, wk, wv, wo, _trace=False, _debug_taps=False,
           _warmup=False):
    in_maps = _host_prep(x, freqs_cis, wq, wk, wv, wo)
    nc = get_nc(_debug_taps)
    if _warmup:
        run_bass_kernel_spmd(
            nc, in_maps, core_ids=list(range(N_CORES)), trace=False
        )
    res = run_bass_kernel_spmd(
        nc, in_maps, core_ids=list(range(N_CORES)), trace=_trace
    )
    full = np.concatenate([res.results[c]["out"] for c in range(N_CORES)], axis=1)
    out = full.reshape(1, S, DIM).astype(np.float32)
    if _trace or _debug_taps:
        kernel.last_results = res
    return out
